# revision 1
# baseline (speedup 1.0000x reference)
"""Trainium2 Bass kernel for nn_AutoPruneNet (MLP policy/baseline heads + sampling).

Math (per row r of TB = T*B rows):
    h1 = relu(x @ W1.T + b1)            x: [512], h1: [400]
    h2 = relu(h1 @ W2.T + b2)           h2: [300]
    core = [h2, clip(reward,-1,1), last_action]   [302]
    pl = sigmoid(core @ Wp.T + bp)      [2]  (mu, sigma)
    baseline = core @ Wb.T + bb         [1]
    action = pl0 + pl1 * eps
    out[r] = [pl0, pl1, baseline, action]

Distribution: pure data parallel, TB rows split contiguously across 8 cores
(16384 rows each); weights replicated.

Device layout: activations stay feature-major ("transposed"): [feature, row],
so the contraction dim of every matmul sits on SBUF partitions and no on-chip
transposes are needed. The host pre-transposes the frame once and the output
back once. SBUF access patterns must start at partition 0/32/64/96, so:
  - the three head outputs are spread to psum partitions 0/32/64 via
    zero-padded head-weight columns, then moved to partition 0 by the ACT
    engine (which tolerates differing in/out partition bases);
  - [clip(reward); last_action] ride at partitions 96/97 of the last fc2
    output chunk (rows 44..95 zeroed), so the head contraction needs no
    extra matmul stream.
"""
import sys
import types

import numpy as np
import ml_dtypes

import concourse.bacc as bacc
import concourse.bass as bass
import concourse.mybir as mybir
import concourse.tile as tile
from concourse.bass import ds, ts
from concourse.bass_utils import run_bass_kernel_spmd


def _install_ntff_hook_shim():
    """Provide the optional antenv.axon_hooks module if the image lacks it,
    so a BASS_TRACE env var in the caller can't crash run_bass_kernel_spmd.
    Registers the real NTFF profile hook when the axon .so supports it."""
    try:
        import antenv.axon_hooks  # noqa: F401
        return
    except Exception:
        pass
    try:
        import antenv
    except Exception:
        return
    mod = types.ModuleType("antenv.axon_hooks")
    state = {"hook": None}
    mod.set_axon_ntff_profile_hook = lambda h: state.__setitem__("hook", h)
    mod.get_axon_ntff_profile_hook = lambda: state["hook"]
    sys.modules["antenv.axon_hooks"] = mod
    antenv.axon_hooks = mod
    try:
        from trn_agent_boot.trn_boot import _ntff_profile_via_ctypes
        mod.set_axon_ntff_profile_hook(
            _ntff_profile_via_ctypes('/opt/axon/libaxon_pjrt.so'))
    except Exception:
        pass


_install_ntff_hook_shim()

BF16 = ml_dtypes.bfloat16

N_CORES = 8
T, B, OBS = 64, 2048, 512
H1, H2 = 400, 300
TB = T * B
R = TB // N_CORES       # rows per core
NT = 512                # rows per row-tile (matmul moving dim)
OG = 4                  # row-tiles per output-DMA group

F32 = mybir.dt.float32
BF = mybir.dt.bfloat16
AF = mybir.ActivationFunctionType
ALU = mybir.AluOpType

# fc1 output (h1) chunking (also fc2 contraction chunking)
M1 = [(0, 100), (100, 100), (200, 100), (300, 100)]
# fc2 output (h2) chunking: {128, 128, 44}; chunk 2 padded to 98 rows with
# zeros at 44..95 and [cr; la] at 96..97
M2 = [(0, 128), (128, 128), (256, 44)]


def build_bass(rows: int):
    """Build the per-core Bass program for `rows` rows (rows % (NT*OG) == 0)."""
    assert rows % (NT * OG) == 0
    n_tiles = rows // NT

    nc = bacc.Bacc("TRN2", target_bir_lowering=False, debug=False)

    xt_d = nc.dram_tensor("xt", [128, 4, rows], BF, kind="ExternalInput")
    rwla_d = nc.dram_tensor("rwla", [2, rows], BF, kind="ExternalInput")
    eps_d = nc.dram_tensor("eps", [1, rows], F32, kind="ExternalInput")
    w1_d = nc.dram_tensor("w1", [128, 4, 400], BF, kind="ExternalInput")
    w2_d = nc.dram_tensor("w2", [100, 4, 300], BF, kind="ExternalInput")
    wh_d = nc.dram_tensor("wh", [128, 3, 65], BF, kind="ExternalInput")
    b1_d = nc.dram_tensor("b1", [100, 4], F32, kind="ExternalInput")
    b2_d = nc.dram_tensor("b2", [128, 3], F32, kind="ExternalInput")
    bh_d = nc.dram_tensor("bh", [65, 1], F32, kind="ExternalInput")
    out_d = nc.dram_tensor("out", [4, rows], F32, kind="ExternalOutput")

    with tile.TileContext(nc) as tc:
        with (
            tc.tile_pool(name="w", bufs=1) as wpool,
            tc.tile_pool(name="x", bufs=3) as xpool,
            tc.tile_pool(name="h1", bufs=8) as h1pool,
            tc.tile_pool(name="core", bufs=8) as cpool,
            tc.tile_pool(name="s", bufs=4) as spool,
            tc.tile_pool(name="ob", bufs=2) as opool,
            tc.tile_pool(name="ps1", bufs=4, space="PSUM") as ppool1,
            tc.tile_pool(name="ps2", bufs=2, space="PSUM") as ppool2,
            tc.tile_pool(name="ps3", bufs=2, space="PSUM") as ppool3,
        ):
            w1_sb = wpool.tile([128, 4, 400], BF, tag="w1")
            nc.scalar.dma_start(w1_sb[:], w1_d[:])
            w2_sb = wpool.tile([100, 4, 300], BF, tag="w2")
            nc.scalar.dma_start(w2_sb[:], w2_d[:])
            wh_sb = wpool.tile([128, 3, 65], BF, tag="wh")
            nc.scalar.dma_start(wh_sb[:], wh_d[:])
            b1_sb = wpool.tile([100, 4, 1], F32, tag="b1")
            nc.scalar.dma_start(b1_sb[:], b1_d[:])
            b2_sb = wpool.tile([128, 3, 1], F32, tag="b2")
            nc.scalar.dma_start(b2_sb[:], b2_d[:])
            bh_sb = wpool.tile([65, 1], F32, tag="bh")
            nc.scalar.dma_start(bh_sb[:], bh_d[:])

            # Software pipeline: the head matmuls + epilogue of tile t-1 are
            # emitted between fc1(t) and fc2(t), so fc2's matmuls get three
            # extra streams of slack for the fc1 relus to land (profiling
            # showed fc2 stalling ~1.2us on the relu semaphore otherwise).
            obs = {}        # group -> (ob tile, et8 tile)
            pending = None  # (cores, t) awaiting head + epilogue

            def emit_head_epilogue(cores, t):
                g, ti = divmod(t, OG)
                ob, et8 = obs[g]
                # heads: psum rows 0=mu_pre, 32=sigma_pre, 64=baseline_pre
                psh = ppool3.tile([65, NT], F32, tag="ps3")
                nc.tensor.matmul(psh[:], wh_sb[0:98, 2, :], cores[2][:],
                                 start=True, stop=False)
                nc.tensor.matmul(psh[:], wh_sb[:, 0, :], cores[0][:],
                                 start=False, stop=False)
                nc.tensor.matmul(psh[:], wh_sb[:, 1, :], cores[1][:],
                                 start=False, stop=True)
                # epilogue — ACT moves rows 32/64 down to partition 0
                sl = ds(ti * NT, NT)
                nc.scalar.activation(ob[:, 0, sl], psh[0:1, :],
                                     AF.Sigmoid, bias=bh_sb[0:1, :])
                nc.scalar.activation(ob[:, 1, sl], psh[32:33, :],
                                     AF.Sigmoid, bias=bh_sb[32:33, :])
                nc.scalar.activation(ob[:, 2, sl], psh[64:65, :],
                                     AF.Identity, bias=bh_sb[64:65, :])
                se = spool.tile([1, NT], F32, tag="se")
                nc.vector.tensor_mul(se[:], ob[:, 1, sl], et8[:, sl])
                nc.vector.tensor_add(ob[:, 3, sl], ob[:, 0, sl], se[:])
                if ti == OG - 1:
                    nc.sync.dma_start(out_d[0:4, ts(g, OG * NT)], ob[:])
                    del obs[g]

            for t in range(n_tiles + 1):
                h1s = None
                if t < n_tiles:
                    g = t // OG
                    if t % OG == 0:
                        # output buffer for this group of row-tiles
                        # (rows: pl0, pl1, baseline, action — at partition 0)
                        ob = opool.tile([1, 4, OG * NT], F32, tag="ob")
                        et8 = opool.tile([1, OG * NT], F32, tag="eps8")
                        nc.sync.dma_start(et8[:],
                                          eps_d[:, ts(g, OG * NT)])
                        obs[g] = (ob, et8)
                    xt_t = xpool.tile([128, 4, NT], BF, tag="xt")
                    nc.sync.dma_start(xt_t[:], xt_d[:, :, ts(t, NT)])

                    # fc1: h1T chunks of 100
                    h1s = []
                    for m, (m0, mw) in enumerate(M1):
                        ps = ppool1.tile([mw, NT], F32, tag="ps1")
                        for k in range(4):
                            nc.tensor.matmul(
                                ps[:],
                                w1_sb[:, k, ds(m0, mw)],
                                xt_t[:, k, :],
                                start=(k == 0),
                                stop=(k == 3),
                            )
                        hs = h1pool.tile([mw, NT], BF, tag=f"h1_{m}",
                                         name=f"h1_{m}")
                        # relu(psum + b1) on DVE: (in + bias) max 0
                        nc.vector.tensor_scalar(
                            hs[:], ps[:], b1_sb[0:mw, m, :], 0.0,
                            ALU.add, ALU.max
                        )
                        h1s.append(hs)

                if pending is not None:
                    emit_head_epilogue(*pending)
                    pending = None

                if t < n_tiles:
                    # fc2: h2T chunks {128, 128, 44+zeros+[cr;la]}; the m=2
                    # chunk goes first so its multi-engine assembly (relu +
                    # rwla DMA + clip + memsets) finishes before the head
                    # matmuls consume it
                    cores = [None, None, None]
                    for m in (2, 0, 1):
                        m0, mw = M2[m]
                        ps2 = ppool2.tile([mw, NT], F32, tag="ps2")
                        for k in range(4):
                            nc.tensor.matmul(
                                ps2[:],
                                w2_sb[0:M1[k][1], k, ds(m0, mw)],
                                h1s[k][:],
                                start=(k == 0),
                                stop=(k == 3),
                            )
                        if m < 2:
                            cm = cpool.tile([128, NT], BF, tag="c")
                            nc.scalar.activation(cm[:], ps2[:], AF.Relu,
                                                 bias=b2_sb[0:mw, m, :])
                        else:
                            cm = cpool.tile([98, NT], BF, tag="c2")
                            nc.gpsimd.memset(cm[32:64, :], 0.0)
                            nc.gpsimd.memset(cm[64:96, :], 0.0)
                            nc.scalar.activation(cm[0:mw, :], ps2[:], AF.Relu,
                                                 bias=b2_sb[0:mw, m, :])
                            nc.sync.dma_start(cm[96:98, :],
                                              rwla_d[:, ts(t, NT)])
                            nc.vector.tensor_scalar(
                                cm[96:97, :], cm[96:97, :], -1.0, 1.0,
                                ALU.max, ALU.min)
                        cores[m] = cm
                    pending = (cores, t)

    nc.compile()
    return nc


def host_prep(frame, reward, last_action, eps, W1, b1, W2, b2, Wp, bp, Wb, bb,
              rows=R, n_cores=N_CORES):
    """Shard + lay out inputs for the device program. Returns in_maps."""
    frame = np.asarray(frame, np.float32).reshape(TB, OBS)
    reward = np.asarray(reward, np.float32).reshape(TB)
    la = np.asarray(last_action).reshape(TB).astype(BF16)
    eps = np.asarray(eps, np.float32).reshape(TB)

    W1 = np.asarray(W1, np.float32)
    W2 = np.asarray(W2, np.float32)
    b1 = np.asarray(b1, np.float32)
    b2 = np.asarray(b2, np.float32)
    Wp = np.asarray(Wp, np.float32)
    bp = np.asarray(bp, np.float32)
    Wb = np.asarray(Wb, np.float32)
    bb = np.asarray(bb, np.float32)

    w1_h = np.ascontiguousarray(
        W1.T.reshape(4, 128, 400).transpose(1, 0, 2)).astype(BF16)
    w2_h = np.ascontiguousarray(
        W2.T.reshape(4, 100, 300).transpose(1, 0, 2)).astype(BF16)
    # head weights: columns 0/32/64 of a zero-padded 65-wide matrix hold
    # (mu, sigma, baseline); contraction rows follow the fc2 chunking
    # {128, 128, 44} with rows 44..95 zero and [cr; la] weights at 96/97
    Wh65 = np.zeros((302, 65), np.float32)
    Wh65[:, 0] = Wp[0]
    Wh65[:, 32] = Wp[1]
    Wh65[:, 64] = Wb[0]
    wh_h = np.zeros((128, 3, 65), np.float32)
    wh_h[:, 0, :] = Wh65[0:128]
    wh_h[:, 1, :] = Wh65[128:256]
    wh_h[0:44, 2, :] = Wh65[256:300]
    wh_h[96:98, 2, :] = Wh65[300:302]
    wh_h = wh_h.astype(BF16)
    b1_h = np.ascontiguousarray(b1.reshape(4, 100).T)
    b2_h = np.zeros((128, 3), np.float32)
    b2_h[0:128, 0] = b2[0:128]
    b2_h[0:128, 1] = b2[128:256]
    b2_h[0:44, 2] = b2[256:300]
    bh_h = np.zeros((65, 1), np.float32)
    bh_h[0, 0] = bp[0]
    bh_h[32, 0] = bp[1]
    bh_h[64, 0] = bb[0]

    in_maps = []
    for c in range(n_cores):
        sl = slice(c * rows, (c + 1) * rows)
        xt = np.ascontiguousarray(
            frame[sl].T.reshape(4, 128, rows).transpose(1, 0, 2)).astype(BF16)
        rwla = np.stack([reward[sl].astype(BF16), la[sl]], axis=0)
        in_maps.append({
            "xt": xt,
            "rwla": rwla,
            "eps": eps[sl].reshape(1, rows),
            "w1": w1_h, "w2": w2_h, "wh": wh_h,
            "b1": b1_h, "b2": b2_h, "bh": bh_h,
        })
    return in_maps


def assemble_out(per_core_outs):
    """[4, R] per core (rows: pl0, pl1, baseline, action) -> [T, B, 4]."""
    outs = []
    for o in per_core_outs:
        outs.append(np.asarray(o).T.reshape(-1, B, 4))
    return np.ascontiguousarray(
        np.concatenate(outs, axis=0).astype(np.float32))


_NC_CACHE = {}


def kernel(**inputs) -> np.ndarray:
    in_maps = host_prep(**inputs)
    if R not in _NC_CACHE:
        _NC_CACHE[R] = build_bass(R)
    nc = _NC_CACHE[R]
    res = run_bass_kernel_spmd(nc, in_maps, core_ids=list(range(N_CORES)))
    return assemble_out([res.results[c]["out"] for c in range(N_CORES)])



# revision 3
# speedup vs baseline: 1.3772x; 1.3772x over previous
"""Trainium2 Bass kernel for nn_AutoPruneNet (MLP policy/baseline heads + sampling).

Math (per row r of TB = T*B rows):
    h1 = relu(x @ W1.T + b1)            x: [512], h1: [400]
    h2 = relu(h1 @ W2.T + b2)           h2: [300]
    core = [h2, clip(reward,-1,1), last_action]   [302]
    pl = sigmoid(core @ Wp.T + bp)      [2]  (mu, sigma)
    baseline = core @ Wb.T + bb         [1]
    action = pl0 + pl1 * eps
    out[r] = [pl0, pl1, baseline, action]

Distribution: pure data parallel, TB rows split contiguously across 8 cores
(16384 rows each); weights replicated.

Precision: fp8(e4m3) activations + weights with DoubleRow matmuls (2 fp8
weights per PE cell -> K=256 per stream), roughly halving PE streams vs bf16.
Weights are scaled x8 on host so they sit in e4m3's normal range; the scale
compounds through the layers (psum1 = 8*y1, psum2 = 64*y2, psum_h = 64*z) and
is divided back out for free via the ACT engine's input `scale` operand.
Activations are stored as 8*h in fp8 (well within e4m3 range).

Layout: feature-major [feature, row]; contraction dims zero-padded to 512
(fc2) — padding K costs no PE time (stream cost depends only on N=512).
Per row-tile of 512 rows: fc1 = 4 M-chunks x 2 DR streams, fc2 = 3 M-chunks
x 2 DR streams, head = 1 DR stream (h2[0:256]) + 1 normal fp8 stream over the
baseline-style [44 x h2 | zeros | cr la] 98-partition chunk. Head outputs land
at psum partitions 0/32/64 via zero-padded 65-wide head weights; ACT/DVE move
them to partition 0 of the output buffer.
"""
import sys
import types

import numpy as np
import ml_dtypes

import concourse.bacc as bacc
import concourse.bass as bass
import concourse.mybir as mybir
import concourse.tile as tile
from concourse.bass import ds, ts
from concourse.bass_utils import run_bass_kernel_spmd


def _install_ntff_hook_shim():
    """Provide the optional antenv.axon_hooks module if the image lacks it,
    so a BASS_TRACE env var in the caller can't crash run_bass_kernel_spmd.
    Registers the real NTFF profile hook when the axon .so supports it."""
    try:
        import antenv.axon_hooks  # noqa: F401
        return
    except Exception:
        pass
    try:
        import antenv
    except Exception:
        return
    mod = types.ModuleType("antenv.axon_hooks")
    state = {"hook": None}
    mod.set_axon_ntff_profile_hook = lambda h: state.__setitem__("hook", h)
    mod.get_axon_ntff_profile_hook = lambda: state["hook"]
    sys.modules["antenv.axon_hooks"] = mod
    antenv.axon_hooks = mod
    try:
        from trn_agent_boot.trn_boot import _ntff_profile_via_ctypes
        mod.set_axon_ntff_profile_hook(
            _ntff_profile_via_ctypes('/opt/axon/libaxon_pjrt.so'))
    except Exception:
        pass


_install_ntff_hook_shim()

E4 = ml_dtypes.float8_e4m3fn

N_CORES = 8
T, B, OBS = 64, 2048, 512
H1, H2 = 400, 300
TB = T * B
R = TB // N_CORES       # rows per core
NT = 512                # rows per row-tile (matmul moving dim)
OG = 4                  # row-tiles per output-DMA group

F32 = mybir.dt.float32
F8 = mybir.dt.float8e4
AF = mybir.ActivationFunctionType
ALU = mybir.AluOpType
DR = mybir.MatmulPerfMode.DoubleRow

# fc1 output (h1) chunking: {128,128,128,32}; last chunk is 16 real rows of
# h1 plus 16 zero-pad rows (weights zero) so its sbuf destination starts the
# j=1 half of h1b at partition 0 and the once-memset zero region can start
# at partition 32.
M1 = [(0, 128), (128, 128), (256, 128), (384, 32)]
# fc2 output (h2) chunking: {128, 128, 44}
M2 = [(0, 128), (128, 128), (256, 44)]


def build_bass(rows: int):
    """Build the per-core Bass program for `rows` rows (rows % (NT*OG) == 0)."""
    assert rows % (NT * OG) == 0
    n_tiles = rows // NT

    nc = bacc.Bacc("TRN2", target_bir_lowering=False, debug=False)

    xt_d = nc.dram_tensor("xt", [128, 2, 2, rows], F8, kind="ExternalInput")
    rwla_d = nc.dram_tensor("rwla", [2, rows], F8, kind="ExternalInput")
    eps_d = nc.dram_tensor("eps", [1, rows], F32, kind="ExternalInput")
    w1_d = nc.dram_tensor("w1", [128, 2, 2, 416], F8, kind="ExternalInput")
    w2_d = nc.dram_tensor("w2", [128, 2, 2, 304], F8, kind="ExternalInput")
    wh1_d = nc.dram_tensor("wh1", [128, 2, 80], F8, kind="ExternalInput")
    wh2_d = nc.dram_tensor("wh2", [98, 80], F8, kind="ExternalInput")
    b1_d = nc.dram_tensor("b1", [128, 4], F32, kind="ExternalInput")
    b2_d = nc.dram_tensor("b2", [128, 3], F32, kind="ExternalInput")
    bh_d = nc.dram_tensor("bh", [65, 1], F32, kind="ExternalInput")
    out_d = nc.dram_tensor("out", [4, rows], F32, kind="ExternalOutput")

    with tile.TileContext(nc) as tc:
        with (
            tc.tile_pool(name="w", bufs=1) as wpool,
            tc.tile_pool(name="x", bufs=3) as xpool,
            tc.tile_pool(name="h1a", bufs=3) as h1apool,
            tc.tile_pool(name="c1", bufs=3) as c1pool,
            tc.tile_pool(name="s", bufs=4) as spool,
            tc.tile_pool(name="ob", bufs=2) as opool,
            tc.tile_pool(name="ps1", bufs=4, space="PSUM") as ppool1,
            tc.tile_pool(name="ps2", bufs=2, space="PSUM") as ppool2,
            tc.tile_pool(name="ps3", bufs=2, space="PSUM") as ppool3,
        ):
            w1_sb = wpool.tile([128, 2, 2, 416], F8, tag="w1")
            nc.scalar.dma_start(w1_sb[:], w1_d[:])
            w2_sb = wpool.tile([128, 2, 2, 304], F8, tag="w2")
            nc.scalar.dma_start(w2_sb[:], w2_d[:])
            wh1_sb = wpool.tile([128, 2, 80], F8, tag="wh1")
            nc.scalar.dma_start(wh1_sb[:], wh1_d[:])
            wh2_sb = wpool.tile([98, 80], F8, tag="wh2")
            nc.scalar.dma_start(wh2_sb[:], wh2_d[:])
            b1_sb = wpool.tile([128, 4, 1], F32, tag="b1")
            nc.scalar.dma_start(b1_sb[:], b1_d[:])
            b2_sb = wpool.tile([128, 3, 1], F32, tag="b2")
            nc.scalar.dma_start(b2_sb[:], b2_d[:])
            bh_sb = wpool.tile([65, 1], F32, tag="bh")
            nc.scalar.dma_start(bh_sb[:], bh_d[:])

            # Persistent ping-pong buffers whose zero regions are memset ONCE:
            #  h1b: j=0 -> h1 chunk2 (rewritten each tile); j=1 partitions
            #       0..31 -> h1 chunk3 (rewritten; rows 16..31 are zero via
            #       zero weights); j=1 partitions 32..127 -> zero forever.
            #  c2:  [44 x h2 chunk | zeros 44..95 | cr la at 96/97]; only
            #       [0:44] and [96:98] are rewritten per tile.
            NBUF = 3
            h1b_bufs, c2_bufs = [], []
            for i in range(NBUF):
                hb = wpool.tile([128, 2, NT], F8, tag=f"h1b{i}")
                nc.gpsimd.memset(hb[32:64, 1, :], 0.0)
                nc.gpsimd.memset(hb[64:128, 1, :], 0.0)
                h1b_bufs.append(hb)
                cb = wpool.tile([98, NT], F8, tag=f"c2{i}")
                nc.gpsimd.memset(cb[32:64, :], 0.0)
                nc.gpsimd.memset(cb[64:96, :], 0.0)
                c2_bufs.append(cb)

            # Software pipeline: the head matmuls + epilogue of tile t-1 are
            # emitted between fc1(t) and fc2(t) so fc2's matmuls give the fc1
            # relus slack to land.
            obs = {}        # group -> (ob tile, et8 tile)
            pending = None  # (c1, c2, t) awaiting head + epilogue

            def emit_head_epilogue(c1, c2, t):
                g, ti = divmod(t, OG)
                ob, et8 = obs[g]
                # heads: psum rows 0=mu_pre, 32=sigma_pre, 64=baseline_pre
                # psum = 64*z  (8*core times 8*Wh)
                psh = ppool3.tile([65, NT], F32, tag="ps3")
                nc.tensor.matmul(psh[:], wh2_sb[:, 0:65], c2[:],
                                 start=True, stop=False)
                nc.tensor.matmul(psh[:], wh1_sb[:, :, 0:65], c1[:],
                                 start=False, stop=True, perf_mode=DR)
                # epilogue — ACT moves rows 32/64 down to partition 0
                sl = ds(ti * NT, NT)
                nc.scalar.activation(ob[:, 0, sl], psh[0:1, :],
                                     AF.Sigmoid, bias=bh_sb[0:1, :],
                                     scale=1.0 / 64.0)
                nc.scalar.activation(ob[:, 1, sl], psh[32:33, :],
                                     AF.Sigmoid, bias=bh_sb[32:33, :],
                                     scale=1.0 / 64.0)
                # baseline on DVE: (psum + 64*bb) * 1/64   (bh_sb[64] = 64*bb)
                nc.vector.tensor_scalar(
                    ob[:, 2, sl], psh[64:65, :], bh_sb[64:65, :], 1.0 / 64.0,
                    ALU.add, ALU.mult)
                se = spool.tile([1, NT], F32, tag="se")
                nc.vector.tensor_mul(se[:], ob[:, 1, sl], et8[:, sl])
                nc.vector.tensor_add(ob[:, 3, sl], ob[:, 0, sl], se[:])
                if ti == OG - 1:
                    nc.sync.dma_start(out_d[0:4, ts(g, OG * NT)], ob[:])
                    del obs[g]

            for t in range(n_tiles + 1):
                h1a = h1b = c2 = None
                if t < n_tiles:
                    g = t // OG
                    if t % OG == 0:
                        # output buffer for this group of row-tiles
                        # (rows: pl0, pl1, baseline, action — at partition 0)
                        ob = opool.tile([1, 4, OG * NT], F32, tag="ob")
                        et8 = opool.tile([1, OG * NT], F32, tag="eps8")
                        nc.sync.dma_start(et8[:],
                                          eps_d[:, ts(g, OG * NT)])
                        obs[g] = (ob, et8)
                    xt_t = xpool.tile([128, 2, 2, NT], F8, tag="xt")
                    nc.sync.dma_start(xt_t[:], xt_d[:, :, :, ts(t, NT)])
                    h1b = h1b_bufs[t % NBUF]
                    c2 = c2_bufs[t % NBUF]
                    nc.sync.dma_start(c2[96:98, :], rwla_d[:, ts(t, NT)])

                    # fc1: h1T chunks {128,128,128,32}; psum = 8*y1
                    h1a = h1apool.tile([128, 2, NT], F8, tag="h1a")
                    for c, (m0, mw) in enumerate(M1):
                        ps = ppool1.tile([128, NT], F32, tag="ps1")
                        for k in range(2):
                            nc.tensor.matmul(
                                ps[0:mw, :],
                                w1_sb[:, k, :, ds(m0, mw)],
                                xt_t[:, k, :, :],
                                start=(k == 0),
                                stop=(k == 1),
                                perf_mode=DR,
                            )
                        # relu((8y1) + 8b1) on DVE -> 8*h1 in fp8
                        if c < 2:
                            dest = h1a[:, c, :]
                        elif c == 2:
                            dest = h1b[:, 0, :]
                        else:
                            dest = h1b[0:32, 1, :]
                        nc.vector.tensor_scalar(
                            dest, ps[0:mw, :], b1_sb[0:mw, c, :], 0.0,
                            ALU.add, ALU.max
                        )

                if pending is not None:
                    emit_head_epilogue(*pending)
                    pending = None

                if t < n_tiles:
                    # fc2: h2T chunks {128, 128, 44}; psum = 64*y2; the m=2
                    # chunk goes first so c2's assembly (relu + rwla DMA)
                    # finishes before the head matmuls consume it
                    c1 = c1pool.tile([128, 2, NT], F8, tag="c1")
                    for m in (2, 0, 1):
                        m0, mw = M2[m]
                        ps2 = ppool2.tile([128, NT], F32, tag="ps2")
                        for k in range(2):
                            rhs = h1a if k == 0 else h1b
                            nc.tensor.matmul(
                                ps2[0:mw, :],
                                w2_sb[:, k, :, ds(m0, mw)],
                                rhs[:, :, :],
                                start=(k == 0),
                                stop=(k == 1),
                                perf_mode=DR,
                            )
                        # relu(64y2/8 + 8b2) on ACT -> 8*h2 in fp8
                        if m < 2:
                            nc.scalar.activation(c1[:, m, :], ps2[0:mw, :],
                                                 AF.Relu,
                                                 bias=b2_sb[0:mw, m, :],
                                                 scale=0.125)
                        else:
                            nc.scalar.activation(c2[0:44, :], ps2[0:mw, :],
                                                 AF.Relu,
                                                 bias=b2_sb[0:mw, m, :],
                                                 scale=0.125)
                    pending = (c1, c2, t)

    nc.compile()
    return nc


def host_prep(frame, reward, last_action, eps, W1, b1, W2, b2, Wp, bp, Wb, bb,
              rows=R, n_cores=N_CORES):
    """Shard + lay out inputs for the device program. Returns in_maps."""
    frame = np.asarray(frame, np.float32).reshape(TB, OBS)
    reward = np.asarray(reward, np.float32).reshape(TB)
    la = np.asarray(last_action).reshape(TB).astype(np.float32)
    eps = np.asarray(eps, np.float32).reshape(TB)

    W1 = np.asarray(W1, np.float32)
    W2 = np.asarray(W2, np.float32)
    b1 = np.asarray(b1, np.float32)
    b2 = np.asarray(b2, np.float32)
    Wp = np.asarray(Wp, np.float32)
    bp = np.asarray(bp, np.float32)
    Wb = np.asarray(Wb, np.float32)
    bb = np.asarray(bb, np.float32)

    # frame features f are split as f = 256k + 128j + ki  ->  [ki, k, j, r]
    frame_q = frame.astype(E4)          # one pass over the big tensor
    W1p = np.zeros((416, 512), np.float32)
    W1p[0:400] = 8.0 * W1
    w1_h = np.ascontiguousarray(
        W1p.T.reshape(2, 2, 128, 416).transpose(2, 0, 1, 3)).astype(E4)
    W2p = np.zeros((304, 512), np.float32)
    W2p[0:300, 0:400] = 8.0 * W2
    w2_h = np.ascontiguousarray(
        W2p.T.reshape(2, 2, 128, 304).transpose(2, 0, 1, 3)).astype(E4)

    # head weights: columns 0/32/64 of a zero-padded 65-wide matrix hold
    # (mu, sigma, baseline); rows follow the core layout
    Wh65 = np.zeros((302, 65), np.float32)
    Wh65[:, 0] = 8.0 * Wp[0]
    Wh65[:, 32] = 8.0 * Wp[1]
    Wh65[:, 64] = 8.0 * Wb[0]
    wh1_h = np.zeros((128, 2, 80), np.float32)
    wh1_h[:, :, 0:65] = Wh65[0:256].reshape(2, 128, 65).transpose(1, 0, 2)
    wh1_h = wh1_h.astype(E4)
    wh2_h = np.zeros((98, 80), np.float32)
    wh2_h[0:44, 0:65] = Wh65[256:300]
    wh2_h[96:98, 0:65] = Wh65[300:302]
    wh2_h = wh2_h.astype(E4)

    b1s = np.zeros(512, np.float32)
    b1s[0:400] = 8.0 * b1
    b1_h = np.ascontiguousarray(b1s.reshape(4, 128).T)
    b2s = np.zeros(384, np.float32)
    b2s[0:300] = 8.0 * b2
    b2_h = np.ascontiguousarray(b2s.reshape(3, 128).T)
    bh_h = np.zeros((65, 1), np.float32)
    bh_h[0, 0] = bp[0]
    bh_h[32, 0] = bp[1]
    bh_h[64, 0] = 64.0 * bb[0]

    cr8 = (8.0 * np.clip(reward, -1.0, 1.0)).astype(E4)
    la8 = (8.0 * la).astype(E4)

    in_maps = []
    for c in range(n_cores):
        sl = slice(c * rows, (c + 1) * rows)
        xt = np.ascontiguousarray(
            frame_q[sl].T.reshape(2, 2, 128, rows).transpose(2, 0, 1, 3))
        rwla = np.stack([cr8[sl], la8[sl]], axis=0)
        in_maps.append({
            "xt": xt,
            "rwla": rwla,
            "eps": eps[sl].reshape(1, rows),
            "w1": w1_h, "w2": w2_h, "wh1": wh1_h, "wh2": wh2_h,
            "b1": b1_h, "b2": b2_h, "bh": bh_h,
        })
    return in_maps


def assemble_out(per_core_outs):
    """[4, R] per core (rows: pl0, pl1, baseline, action) -> [T, B, 4]."""
    outs = []
    for o in per_core_outs:
        outs.append(np.asarray(o).T.reshape(-1, B, 4))
    return np.ascontiguousarray(
        np.concatenate(outs, axis=0).astype(np.float32))


_NC_CACHE = {}


def kernel(**inputs) -> np.ndarray:
    in_maps = host_prep(**inputs)
    if R not in _NC_CACHE:
        _NC_CACHE[R] = build_bass(R)
    nc = _NC_CACHE[R]
    res = run_bass_kernel_spmd(nc, in_maps, core_ids=list(range(N_CORES)))
    return assemble_out([res.results[c]["out"] for c in range(N_CORES)])


# revision 6
# speedup vs baseline: 1.8419x; 1.3374x over previous
"""Trainium2 Bass kernel for nn_AutoPruneNet (MLP policy/baseline heads + sampling).

Math (per row r of TB = T*B rows):
    h1 = relu(x @ W1.T + b1)            x: [512], h1: [400]
    h2 = relu(h1 @ W2.T + b2)           h2: [300]
    core = [h2, clip(reward,-1,1), last_action]   [302]
    pl = sigmoid(core @ Wp.T + bp)      [2]  (mu, sigma)
    baseline = core @ Wb.T + bb         [1]
    action = pl0 + pl1 * eps
    out[r] = [pl0, pl1, baseline, action]

Distribution: pure data parallel, TB rows split contiguously across 8 cores
(16384 rows each); weights replicated.

Precision: fp8(e4m3) activations + weights with DoubleRow matmuls (2 fp8
weights per PE cell -> K=256 per stream), roughly halving PE streams vs bf16.
Weights are scaled x8 on host so they sit in e4m3's normal range; the scale
compounds through the layers (psum1 = 8*y1, psum2 = 64*y2, psum_h = 64*z) and
is divided back out for free via the ACT engine's input `scale` operand.
Activations are stored as 8*h in fp8 (well within e4m3 range).

Layout: fc1/fc2 run feature-major [feature, row] (contraction on partitions,
zero-padded to 512 where needed — K padding costs no PE time, stream cost
depends only on N=512). The HEAD runs row-major: lhsT = core slice
[K, 128 rows] (stationary), rhs = head weights [K, 4] (moving), so the head
psum is [128 rows, (mu,sigma,base,pad)] and the whole sampling epilogue is a
handful of partition-parallel [128, 16, *] ops per 4-tile group instead of
one-partition [1,512] ops (which bottlenecked the DVE/ACT engines). Head
biases ride as a constant-8.0 row appended to the rwla DMA (core row 98)
with 8*b head-weight entries, so psum_h already includes 64*b.
"""
import sys
import types

import numpy as np
import ml_dtypes

import concourse.bacc as bacc
import concourse.bass as bass
import concourse.mybir as mybir
import concourse.tile as tile
from concourse.bass import ds, ts
from concourse.bass_utils import run_bass_kernel_spmd


def _install_ntff_hook_shim():
    """Provide the optional antenv.axon_hooks module if the image lacks it,
    so a BASS_TRACE env var in the caller can't crash run_bass_kernel_spmd.
    Registers the real NTFF profile hook when the axon .so supports it."""
    try:
        import antenv.axon_hooks  # noqa: F401
        return
    except Exception:
        pass
    try:
        import antenv
    except Exception:
        return
    mod = types.ModuleType("antenv.axon_hooks")
    state = {"hook": None}
    mod.set_axon_ntff_profile_hook = lambda h: state.__setitem__("hook", h)
    mod.get_axon_ntff_profile_hook = lambda: state["hook"]
    sys.modules["antenv.axon_hooks"] = mod
    antenv.axon_hooks = mod
    try:
        from trn_agent_boot.trn_boot import _ntff_profile_via_ctypes
        mod.set_axon_ntff_profile_hook(
            _ntff_profile_via_ctypes('/opt/axon/libaxon_pjrt.so'))
    except Exception:
        pass


_install_ntff_hook_shim()

E4 = ml_dtypes.float8_e4m3fn

N_CORES = 8
T, B, OBS = 64, 2048, 512
H1, H2 = 400, 300
TB = T * B
R = TB // N_CORES       # rows per core
NT = 512                # rows per row-tile (matmul moving dim)
OG = 4                  # row-tiles per output group
RC = NT // 128          # 128-row head chunks per tile (4)
GC = OG * RC            # head chunks per group (16)

F32 = mybir.dt.float32
F8 = mybir.dt.float8e4
AF = mybir.ActivationFunctionType
ALU = mybir.AluOpType
DR = mybir.MatmulPerfMode.DoubleRow

# fc1 output (h1) chunking: {128,128,128,32}; last chunk is 16 real rows of
# h1 plus 16 zero-pad rows (weights zero) so the once-memset zero region of
# h1b's j=1 half starts at partition 32.
M1 = [(0, 128), (128, 128), (256, 128), (384, 32)]
# fc2 output (h2) chunking: {128, 128, 44}
M2 = [(0, 128), (128, 128), (256, 44)]


def build_bass(rows: int):
    """Build the per-core Bass program for `rows` rows (rows % (NT*OG) == 0)."""
    assert rows % (NT * OG) == 0
    n_tiles = rows // NT
    n_groups = n_tiles // OG

    nc = bacc.Bacc("TRN2", target_bir_lowering=False, debug=False)

    xt_d = nc.dram_tensor("xt", [128, n_tiles, 2, 2, NT], F8,
                          kind="ExternalInput")
    rwla_d = nc.dram_tensor("rwla", [3, rows], F8, kind="ExternalInput")
    eps_d = nc.dram_tensor("eps", [128, rows // 128], F32,
                           kind="ExternalInput")
    w1_d = nc.dram_tensor("w1", [128, 2, 2, 416], F8, kind="ExternalInput")
    w2_d = nc.dram_tensor("w2", [128, 2, 2, 304], F8, kind="ExternalInput")
    wh1_d = nc.dram_tensor("wh1", [128, 2, 4], F8, kind="ExternalInput")
    wh2_d = nc.dram_tensor("wh2", [99, 4], F8, kind="ExternalInput")
    b1_d = nc.dram_tensor("b1", [128, 4], F32, kind="ExternalInput")
    b2_d = nc.dram_tensor("b2", [128, 3], F32, kind="ExternalInput")
    out_d = nc.dram_tensor("out", [128, rows // 128, 4], F32,
                           kind="ExternalOutput")

    with tile.TileContext(nc) as tc:
        with (
            tc.tile_pool(name="w", bufs=1) as wpool,
            tc.tile_pool(name="x", bufs=3) as xpool,
            tc.tile_pool(name="h1a", bufs=3) as h1apool,
            tc.tile_pool(name="c1", bufs=6) as c1pool,
            tc.tile_pool(name="ot", bufs=2) as opool,
            tc.tile_pool(name="s", bufs=2) as spool,
            tc.tile_pool(name="ps1", bufs=4, space="PSUM") as ppool1,
            tc.tile_pool(name="ps2", bufs=2, space="PSUM") as ppool2,
            tc.tile_pool(name="ps3", bufs=2, space="PSUM") as ppool3,
        ):
            w1_sb = wpool.tile([128, 2, 2, 416], F8, tag="w1")
            nc.scalar.dma_start(w1_sb[:], w1_d[:])
            w2_sb = wpool.tile([128, 2, 2, 304], F8, tag="w2")
            nc.scalar.dma_start(w2_sb[:], w2_d[:])
            wh1_sb = wpool.tile([128, 2, 4], F8, tag="wh1")
            nc.scalar.dma_start(wh1_sb[:], wh1_d[:])
            wh2_sb = wpool.tile([99, 4], F8, tag="wh2")
            nc.scalar.dma_start(wh2_sb[:], wh2_d[:])
            b1_sb = wpool.tile([128, 4, 1], F32, tag="b1")
            nc.scalar.dma_start(b1_sb[:], b1_d[:])
            b2_sb = wpool.tile([128, 3, 1], F32, tag="b2")
            nc.scalar.dma_start(b2_sb[:], b2_d[:])
            eps_sb = wpool.tile([128, rows // 128], F32, tag="eps")
            nc.scalar.dma_start(eps_sb[:], eps_d[:])

            # Persistent rotating buffers whose zero regions are memset ONCE:
            #  h1b: j=0 -> h1 chunk2 (rewritten each tile); j=1 partitions
            #       0..31 -> h1 chunk3 (rewritten; rows 16..31 zero via zero
            #       weights); j=1 partitions 32..127 -> zero forever.
            #  c2:  99 partitions: [0:44] h2 chunk (rewritten), [44:96] zero
            #       forever, [96:99] (cr, la, const-8) DMA'd each tile.
            NB1 = 3
            NB2 = 6
            h1b_bufs, c2_bufs = [], []
            for i in range(NB1):
                hb = wpool.tile([128, 2, NT], F8, tag=f"h1b{i}")
                nc.gpsimd.memset(hb[32:64, 1, :], 0.0)
                nc.gpsimd.memset(hb[64:128, 1, :], 0.0)
                h1b_bufs.append(hb)
            for i in range(NB2):
                cb = wpool.tile([99, NT], F8, tag=f"c2{i}")
                nc.gpsimd.memset(cb[32:64, :], 0.0)
                nc.gpsimd.memset(cb[64:96, :], 0.0)
                c2_bufs.append(cb)

            # group g -> [(c1, c2) per tile-phase], psh, obt
            gtiles = {}
            gps = {}

            def emit_head_phase(g, b):
                """Emit head chunks 4b..4b+3 of group g (rows of its tile
                phase b); after phase 3, the sampling epilogue + out DMA."""
                c1, c2 = gtiles[g][b]
                if b == 0:
                    gps[g] = (ppool3.tile([128, GC, 4], F32, tag="ps3",
                                          name="psh"),
                              opool.tile([128, GC, 4], F32, tag="obt",
                                         name="obt"))
                psh, obt = gps[g]
                for q in range(RC):
                    c = RC * b + q
                    rsl = ds(q * 128, 128)
                    nc.tensor.matmul(psh[:, c, :], c2[:, rsl], wh2_sb[:],
                                     start=True, stop=False)
                    nc.tensor.matmul(psh[:, c, :], c1[:, 0, rsl],
                                     wh1_sb[:, 0, :], start=False, stop=False)
                    nc.tensor.matmul(psh[:, c, :], c1[:, 1, rsl],
                                     wh1_sb[:, 1, :], start=False, stop=True)
                if b == OG - 1:
                    # psum = 64*(z + b);  pl = sigmoid(z + b) etc.
                    nc.scalar.activation(obt[:, :, 0:2], psh[:, :, 0:2],
                                         AF.Sigmoid, scale=1.0 / 64.0)
                    nc.vector.tensor_scalar_mul(obt[:, :, 2], psh[:, :, 2],
                                                1.0 / 64.0)
                    se = spool.tile([128, GC], F32, tag="se")
                    nc.vector.tensor_mul(se[:], obt[:, :, 1],
                                         eps_sb[:, ds(g * GC, GC)])
                    nc.vector.tensor_add(obt[:, :, 3], obt[:, :, 0], se[:])
                    nc.sync.dma_start(out_d[:, ds(g * GC, GC), :], obt[:])
                    del gtiles[g], gps[g]

            for t in range(n_tiles + OG):
                h1a = h1b = c2 = None
                if t < n_tiles:
                    g = t // OG
                    xt_t = xpool.tile([128, 2, 2, NT], F8, tag="xt")
                    nc.sync.dma_start(xt_t[:], xt_d[:, t, :, :, :])
                    h1b = h1b_bufs[t % NB1]
                    c2 = c2_bufs[t % NB2]
                    nc.sync.dma_start(c2[96:99, :], rwla_d[:, ts(t, NT)])

                    # fc1: h1T chunks {128,128,128,32}; psum = 8*y1
                    h1a = h1apool.tile([128, 2, NT], F8, tag="h1a")
                    for c, (m0, mw) in enumerate(M1):
                        ps = ppool1.tile([128, NT], F32, tag="ps1")
                        for k in range(2):
                            nc.tensor.matmul(
                                ps[0:mw, :],
                                w1_sb[:, k, :, ds(m0, mw)],
                                xt_t[:, k, :, :],
                                start=(k == 0),
                                stop=(k == 1),
                                perf_mode=DR,
                            )
                        # relu((8y1) + 8b1) on DVE -> 8*h1 in fp8
                        if c < 2:
                            dest = h1a[:, c, :]
                        elif c == 2:
                            dest = h1b[:, 0, :]
                        else:
                            dest = h1b[0:32, 1, :]
                        nc.vector.tensor_scalar(
                            dest, ps[0:mw, :], b1_sb[0:mw, c, :], 0.0,
                            ALU.add, ALU.max
                        )

                # head of the previous group, interleaved phase by phase
                if t >= OG:
                    emit_head_phase((t - OG) // OG, t % OG)

                if t < n_tiles:
                    # fc2: h2T chunks {128, 128, 44}; psum = 64*y2; the m=2
                    # chunk goes first so c2's assembly (relu + rwla DMA)
                    # finishes before the head matmuls consume it
                    c1 = c1pool.tile([128, 2, NT], F8, tag="c1")
                    for m in (2, 0, 1):
                        m0, mw = M2[m]
                        ps2 = ppool2.tile([128, NT], F32, tag="ps2")
                        for k in range(2):
                            rhs = h1a if k == 0 else h1b
                            nc.tensor.matmul(
                                ps2[0:mw, :],
                                w2_sb[:, k, :, ds(m0, mw)],
                                rhs[:, :, :],
                                start=(k == 0),
                                stop=(k == 1),
                                perf_mode=DR,
                            )
                        # relu(64y2/8 + 8b2) on ACT -> 8*h2 in fp8
                        if m < 2:
                            nc.scalar.activation(c1[:, m, :], ps2[0:mw, :],
                                                 AF.Relu,
                                                 bias=b2_sb[0:mw, m, :],
                                                 scale=0.125)
                        else:
                            nc.scalar.activation(c2[0:44, :], ps2[0:mw, :],
                                                 AF.Relu,
                                                 bias=b2_sb[0:mw, m, :],
                                                 scale=0.125)
                    gtiles.setdefault(t // OG, []).append((c1, c2))

    nc.compile()
    return nc


def host_prep(frame, reward, last_action, eps, W1, b1, W2, b2, Wp, bp, Wb, bb,
              rows=R, n_cores=N_CORES):
    """Shard + lay out inputs for the device program. Returns in_maps."""
    frame = np.asarray(frame, np.float32).reshape(TB, OBS)
    reward = np.asarray(reward, np.float32).reshape(TB)
    la = np.asarray(last_action).reshape(TB).astype(np.float32)
    eps = np.asarray(eps, np.float32).reshape(TB)
    n_tiles = rows // NT

    W1 = np.asarray(W1, np.float32)
    W2 = np.asarray(W2, np.float32)
    b1 = np.asarray(b1, np.float32)
    b2 = np.asarray(b2, np.float32)
    Wp = np.asarray(Wp, np.float32)
    bp = np.asarray(bp, np.float32)
    Wb = np.asarray(Wb, np.float32)
    bb = np.asarray(bb, np.float32)

    # frame features f are split as f = 256k + 128j + ki
    frame_q = frame.astype(E4)          # one pass over the big tensor
    W1p = np.zeros((416, 512), np.float32)
    W1p[0:400] = 8.0 * W1
    w1_h = np.ascontiguousarray(
        W1p.T.reshape(2, 2, 128, 416).transpose(2, 0, 1, 3)).astype(E4)
    W2p = np.zeros((304, 512), np.float32)
    W2p[0:300, 0:400] = 8.0 * W2
    w2_h = np.ascontiguousarray(
        W2p.T.reshape(2, 2, 128, 304).transpose(2, 0, 1, 3)).astype(E4)

    # head weights, row-major heads: columns (mu, sigma, baseline, pad);
    # core rows: 0..255 (c1: f = 128j + ki), then c2 rows {0..43: h2
    # 256..299, 44..95: zero, 96: cr, 97: la, 98: const-8 bias row}
    Wh = np.concatenate([Wp, Wb], axis=0)           # [3, 302]
    bh = np.array([bp[0], bp[1], bb[0]], np.float32)
    wh1_h = np.zeros((128, 2, 4), np.float32)
    wh1_h[:, :, 0:3] = (8.0 * Wh[:, 0:256]).T.reshape(2, 128, 3).transpose(
        1, 0, 2)
    wh1_h = wh1_h.astype(E4)
    wh2_h = np.zeros((99, 4), np.float32)
    wh2_h[0:44, 0:3] = 8.0 * Wh[:, 256:300].T
    wh2_h[96:98, 0:3] = 8.0 * Wh[:, 300:302].T
    wh2_h[98, 0:3] = 8.0 * bh
    wh2_h = wh2_h.astype(E4)

    b1s = np.zeros(512, np.float32)
    b1s[0:400] = 8.0 * b1
    b1_h = np.ascontiguousarray(b1s.reshape(4, 128).T)
    b2s = np.zeros(384, np.float32)
    b2s[0:300] = 8.0 * b2
    b2_h = np.ascontiguousarray(b2s.reshape(3, 128).T)

    cr8 = (8.0 * np.clip(reward, -1.0, 1.0)).astype(E4)
    la8 = (8.0 * la).astype(E4)
    ones8 = np.full(TB, 8.0, np.float32).astype(E4)

    in_maps = []
    for c in range(n_cores):
        sl = slice(c * rows, (c + 1) * rows)
        xt = np.ascontiguousarray(
            frame_q[sl].T.reshape(2, 2, 128, n_tiles, NT)
            .transpose(2, 3, 0, 1, 4))
        rwla = np.stack([cr8[sl], la8[sl], ones8[sl]], axis=0)
        # eps row r lives at [r % 128, r // 128]
        eps_c = np.ascontiguousarray(
            eps[sl].reshape(rows // 128, 128).T)
        in_maps.append({
            "xt": xt,
            "rwla": rwla,
            "eps": eps_c,
            "w1": w1_h, "w2": w2_h, "wh1": wh1_h, "wh2": wh2_h,
            "b1": b1_h, "b2": b2_h,
        })
    return in_maps


def assemble_out(per_core_outs):
    """[128, R//128, 4] per core (row r at [r%128, r//128]) -> [T, B, 4]."""
    outs = []
    for o in per_core_outs:
        o = np.asarray(o)
        outs.append(o.transpose(1, 0, 2).reshape(-1, B, 4))
    return np.ascontiguousarray(
        np.concatenate(outs, axis=0).astype(np.float32))


_NC_CACHE = {}


def kernel(**inputs) -> np.ndarray:
    in_maps = host_prep(**inputs)
    if R not in _NC_CACHE:
        _NC_CACHE[R] = build_bass(R)
    nc = _NC_CACHE[R]
    res = run_bass_kernel_spmd(nc, in_maps, core_ids=list(range(N_CORES)))
    return assemble_out([res.results[c]["out"] for c in range(N_CORES)])
